# revision 11
# baseline (speedup 1.0000x reference)
"""CoAttentionFusion Trainium2 kernel (bf16 chain + fp8-DR weighted sums).

Full-input contract: kernel(**inputs) takes the complete (unsharded) numpy
inputs and returns (out_p, out_s) matching the fp32 reference. Internally
shards batch 16 -> 2 per core across 8 NeuronCores (weights replicated),
builds one SPMD Bass program, and runs it via run_bass_kernel_spmd.

Math per batch b (L1=L2=512, D=512, H=8, HD=128):
  aff_h = tanh(P @ W_aff[h] @ S^T) * (pm_i * sm_j)
  pp = (P @ W_p) head-split; ps = (S @ W_s) head-split
  wp_h = relu(aff_h^T @ pp_h)  -> pool_p = max_h wp_h   (B, L2, HD)
  ws_h = relu(aff_h  @ ps_h)  -> pool_s = max_h ws_h   (B, L1, HD)
  out_p = relu([P, pool_s] @ W_fp + b_fp)
  out_s = relu([S, pool_p] @ W_fs + b_fs)

Precision: the 2e-2 absmax-rel gate kills fp8 on every 512-deep
contraction whose operands carry independent quantization noise (each
such stage alone contributes ~1-3e-2 through the tanh band's sqrt(K)
amplification and the 5-sigma tail of the 4M-element max statistic).
The one stage where fp8 is free is A itself: 92% of A is tanh-saturated
to exactly +-1, so quantizing A (and pp/ps, whose error is not
chain-amplified) costs ~9e-3 total. Hence: everything bf16 except the
weighted sums, which run fp8e4 DoubleRow (2x rate, K=256/instruction).

Perf structure vs the 228us bf16 baseline:
 - Host pre-transposes P/S to (D, L) bf16: inputs DMA straight into lhsT
   layout (no XBAR input transposes, no PE input transposes).
 - A^T via XBAR dma_start_transpose SBUF->SBUF (idle DMA engines)
   instead of 256 PE transposes (saves ~14us PE + their PSUM drains).
 - pool_p/pool_s max-accumulate into slot 4 of the 5-slot concat tiles,
   landing exactly in the final matmuls' lhsT layout.
 - PSUM drains balanced: scalar = tanh + half PW drains + output relus,
   vector = other PW half + projections + pools + fp8 casts of A/A^T.
 - Heads software-pipelined two deep: PW(h) | A(h-1) | WP/WS(h-2).
"""

import numpy as np

import concourse.bacc as bacc
import concourse.mybir as mybir
import concourse.tile as tile
from concourse import bass_utils
from concourse.masks import make_identity

# Problem constants (hardcoded per contract).
B = 16
L = 512  # L1 == L2
D = 512
H = 8
INNER = 1024
HD = INNER // H  # 128
CONCAT = D + HD  # 640
P = 128
NT = L // P  # 4
NCT = CONCAT // P  # 5
NCORES = 8
BPC = B // NCORES  # batches per core

F32 = mybir.dt.float32
BF16 = mybir.dt.bfloat16
FP8 = mybir.dt.float8e4
DR = mybir.MatmulPerfMode.DoubleRow

TANH = mybir.ActivationFunctionType.Tanh
RELU = mybir.ActivationFunctionType.Relu


def _build_program(masks_trivial: bool, bias_trivial: bool):
    nc = bacc.Bacc(
        "TRN2",
        target_bir_lowering=False,
        debug=False,
        enable_asserts=False,
        num_devices=NCORES,
    )

    def din(name, shape, dt=F32):
        return nc.dram_tensor(name, list(shape), dt, kind="ExternalInput").ap()

    primary_t = din("primary_t", (BPC, D, L), BF16)
    secondary_t = din("secondary_t", (BPC, D, L), BF16)
    pmask = din("primary_mask", (BPC, L))
    smask = din("secondary_mask", (BPC, L))
    w_aff = din("W_aff", (H, D, D), BF16)
    w_p = din("W_p", (D, INNER), BF16)
    w_s = din("W_s", (D, INNER), BF16)
    w_fp = din("W_fp", (CONCAT, D), BF16)
    b_fp = din("b_fp", (D,))
    w_fs = din("W_fs", (CONCAT, D), BF16)
    b_fs = din("b_fs", (D,))
    out_p = nc.dram_tensor("out_p", [BPC, L, D], BF16, kind="ExternalOutput").ap()
    out_s = nc.dram_tensor("out_s", [BPC, L, D], BF16, kind="ExternalOutput").ap()

    with tile.TileContext(nc) as tc:
        _body(
            tc,
            primary_t, secondary_t, pmask, smask,
            w_aff, w_p, w_s, w_fp, b_fp, w_fs, b_fs,
            out_p, out_s,
            masks_trivial, bias_trivial,
        )
    nc.compile()
    return nc


def _body(
    tc,
    primary_t, secondary_t, pmask, smask,
    w_aff, w_p, w_s, w_fp, b_fp, w_fs, b_fs,
    out_p, out_s,
    masks_trivial, bias_trivial,
):
    nc = tc.nc

    with (
        tc.tile_pool(name="consts", bufs=1) as consts,
        tc.tile_pool(name="wpool", bufs=1) as wpool,
        tc.tile_pool(name="bpool", bufs=2) as bpool,
        tc.tile_pool(name="hpool", bufs=3) as hpool,
        tc.tile_pool(name="iopool", bufs=4) as iopool,
        tc.tile_pool(name="psum", bufs=2, space="PSUM") as psum,
    ):
        MMB = 6  # psum bufs for matmul accumulation groups

        # PE warm-up transposes: ramp the HAM clock to 8/8 while the first
        # weights stream in.
        ident_w = consts.tile([P, P], BF16, name="ident_w", tag="ident_w")
        make_identity(nc, ident_w)
        for _ in range(20):
            warm = psum.tile([P, P], BF16, name="warm", tag="tr", bufs=2)
            nc.tensor.transpose(warm[:], ident_w[:], ident_w[:])

        # ---- resident weights (bf16, ordered by first use) --------------
        # Three parallel DMA streams so the PE isn't gated on one queue:
        # gpsimd carries W_aff (heads in use order), the scalar queue
        # carries the projection weights, sync carries the final weights.
        w_aff_all = wpool.tile([P, H, NT, D], BF16, name="w_aff_all", tag="w_aff_all")
        for h in range(H):
            nc.gpsimd.dma_start(
                w_aff_all[:, h], w_aff[h].rearrange("(eo p) f -> p eo f", p=P)
            )
        w_p_sb = wpool.tile([P, NT, INNER], BF16, name="w_p_sb", tag="w_p_sb")
        nc.scalar.dma_start(w_p_sb[:], w_p.rearrange("(eo p) d -> p eo d", p=P))
        w_s_sb = wpool.tile([P, NT, INNER], BF16, name="w_s_sb", tag="w_s_sb")
        nc.scalar.dma_start(w_s_sb[:], w_s.rearrange("(eo p) d -> p eo d", p=P))
        w_fp_sb = wpool.tile([P, NCT, D], BF16, name="w_fp_sb", tag="w_fp_sb")
        nc.sync.dma_start(w_fp_sb[:], w_fp.rearrange("(co p) n -> p co n", p=P))
        w_fs_sb = wpool.tile([P, NCT, D], BF16, name="w_fs_sb", tag="w_fs_sb")
        nc.sync.dma_start(w_fs_sb[:], w_fs.rearrange("(co p) n -> p co n", p=P))

        if not bias_trivial:
            bias_p_bc = consts.tile([P, D], F32, name="bias_p_bc", tag="bias_p_bc")
            nc.sync.dma_start(bias_p_bc[:], b_fp.partition_broadcast(P))
            bias_s_bc = consts.tile([P, D], F32, name="bias_s_bc", tag="bias_s_bc")
            nc.sync.dma_start(bias_s_bc[:], b_fs.partition_broadcast(P))

        # pt/st: [e_in, slot, i] bf16, slots 0-3 = P^T/S^T e-blocks, slot 4 =
        # pool (overwritten by the h==0 pool max before any read).
        ptst = {}

        def make_ptst(b):
            if b not in ptst:
                pt = bpool.tile([P, NCT, L], BF16, name="pt", tag="pt")
                st = bpool.tile([P, NCT, L], BF16, name="st", tag="st")
                nc.sync.dma_start(
                    pt[:, 0:NT], primary_t[b].rearrange("(eo p) i -> p eo i", p=P)
                )
                nc.sync.dma_start(
                    st[:, 0:NT], secondary_t[b].rearrange("(fo p) j -> p fo j", p=P)
                )
                ptst[b] = (pt, st)
            return ptst[b]

        make_ptst(0)
        make_ptst(1)

        masks = {}

        def load_masks(b):
            if masks_trivial or b in masks:
                return
            pm_sb = consts.tile([P, NT], F32, name="pm_sb", tag="pm_sb", bufs=2)
            sm_sb = consts.tile([P, NT], F32, name="sm_sb", tag="sm_sb", bufs=2)
            with nc.allow_non_contiguous_dma(reason="tiny 2KB mask load"):
                nc.sync.dma_start(pm_sb[:], pmask[b].rearrange("(io p) -> p io", p=P))
                nc.sync.dma_start(sm_sb[:], smask[b].rearrange("(jo p) -> p jo", p=P))
            pm_bc = consts.tile([P, L], F32, name="pm_bc", tag="pm_bc", bufs=2)
            nc.sync.dma_start(pm_bc[:], pmask[b].partition_broadcast(P))
            sm_bc = consts.tile([P, L], F32, name="sm_bc", tag="sm_bc", bufs=2)
            nc.sync.dma_start(sm_bc[:], smask[b].partition_broadcast(P))
            masks[b] = (pm_sb, sm_sb, pm_bc, sm_bc)

        load_masks(0)

        # ---- pipeline stages, parameterized by (batch, head) ------------
        ppps = {}  # b -> (pp, ps) fp8 projection tiles
        pwts = {}  # (b, h) -> pwt
        heads = {}  # (b, h) -> (a8, at8)

        def proj_stage(b, dh):
            """pp/ps columns dh*512:(dh+1)*512: bf16 matmuls, fp8 stores
            (mask scaling, when present, folds into the drain)."""
            if b not in ppps:
                pp = bpool.tile([P, NT, INNER], FP8, name="pp", tag="pp")
                ps = bpool.tile([P, NT, INNER], FP8, name="ps", tag="ps")
                ppps[b] = (pp, ps)
            pp, ps = ppps[b]
            pt, st = ptst[b]
            load_masks(b)
            for dst, src_t, w_sb, mi in ((pp, pt, w_p_sb, 0), (ps, st, w_s_sb, 1)):
                for io in range(NT):
                    ps_mm = psum.tile(
                        [P, 512], F32, name="ps_proj", tag="mm", bufs=MMB
                    )
                    for eo in range(NT):
                        nc.tensor.matmul(
                            ps_mm[:],
                            src_t[:, eo, io * P:(io + 1) * P],
                            w_sb[:, eo, dh * 512:(dh + 1) * 512],
                            start=(eo == 0),
                            stop=(eo == NT - 1),
                        )
                    dsl = dst[:, io, dh * 512:(dh + 1) * 512]
                    if masks_trivial:
                        nc.vector.tensor_copy(out=dsl, in_=ps_mm[:])
                    else:
                        msb = masks[b][mi]
                        nc.vector.tensor_scalar_mul(dsl, ps_mm[:], msb[:, io:io + 1])

        def stage_pw(b, h):
            """PW^T (f, i) = W_aff[h]^T-contraction with P^T (bf16)."""
            pt, _ = ptst[b]
            pwt = hpool.tile([P, NT, L], BF16, name="pwt", tag="pwt", bufs=2)
            for fo in range(NT):
                ps_mm = psum.tile([P, 512], F32, name="ps_pw", tag="mm", bufs=MMB)
                for eo in range(NT):
                    nc.tensor.matmul(
                        ps_mm[:],
                        w_aff_all[:, h, eo, fo * P:(fo + 1) * P],
                        pt[:, eo, 0:L],
                        start=(eo == 0),
                        stop=(eo == NT - 1),
                    )
                if fo % 2 == 0:
                    nc.scalar.copy(out=pwt[:, fo, :], in_=ps_mm[:])
                else:
                    nc.vector.tensor_copy(out=pwt[:, fo, :], in_=ps_mm[:])
            pwts[(b, h)] = pwt

        def stage_a(b, h):
            """A (i,j) = tanh(PW^T.T @ S^T) -> bf16; fp8 copy a8 (scalar)
            for WP; A^T via XBAR SBUF->SBUF transposes (sync/gpsimd
            queues alternating)."""
            _, st = ptst[b]
            pwt = pwts.pop((b, h))
            a_bf = hpool.tile([P, NT, L], BF16, name="a_bf", tag="a_bf", bufs=3)
            a8 = hpool.tile([P, NT, L], FP8, name="a8", tag="a8", bufs=3)
            at_bf = hpool.tile([P, NT, L], BF16, name="at_bf", tag="at_bf", bufs=3)
            for io in range(NT):
                ps_mm = psum.tile([P, 512], F32, name="ps_a", tag="mm", bufs=MMB)
                for fo in range(NT):
                    nc.tensor.matmul(
                        ps_mm[:],
                        pwt[:, fo, io * P:(io + 1) * P],
                        st[:, fo, 0:L],
                        start=(fo == 0),
                        stop=(fo == NT - 1),
                    )
                nc.scalar.activation(out=a_bf[:, io, :], in_=ps_mm[:], func=TANH)
                nc.scalar.copy(out=a8[:, io, :], in_=a_bf[:, io, :])
                # XBAR: at_bf[pj, jo, io*128+q] = a_bf[q, io, jo*128+pj]
                nc.sync.dma_start_transpose(
                    at_bf[:, :, io * P:(io + 1) * P], a_bf[:, io, :]
                )
            heads[(b, h)] = (a8, at_bf, None)

        def stage_at8(b, h):
            """fp8 cast of A^T (vector, fast SBUF->SBUF)."""
            a8, at_bf, _ = heads[(b, h)]
            at8 = hpool.tile([P, NT, L], FP8, name="at8", tag="at8", bufs=2)
            for c in range(NT):
                nc.vector.tensor_copy(out=at8[:, c, :], in_=at_bf[:, c, :])
            heads[(b, h)] = (a8, at_bf, at8)

        def stage_wp(b, h):
            """wp_h^T (d,j) = pp_h^T-contraction with A (fp8 DR); max-pool
            into st slot 4 (= pool_p, feeds out_s)."""
            pp, _ = ppps[b]
            _, st = ptst[b]
            a8 = heads[(b, h)][0]
            ps_wp = psum.tile([P, L], F32, name="ps_wp", tag="mm", bufs=MMB)
            for k in range(2):
                nc.tensor.matmul(
                    ps_wp[:],
                    pp[:, 2 * k:2 * k + 2, h * HD:(h + 1) * HD],
                    a8[:, 2 * k:2 * k + 2, 0:L],
                    start=(k == 0),
                    stop=(k == 1),
                    perf_mode=DR,
                )
            pool = st[:, NT, :]
            if h == 0:
                nc.vector.tensor_scalar_max(pool, ps_wp[:], 0.0)
            else:
                nc.vector.tensor_max(out=pool, in0=pool, in1=ps_wp[:])

        def stage_ws(b, h):
            """ws_h^T (d,i) = ps_h^T-contraction with A^T (fp8 DR); max-pool
            into pt slot 4 (= pool_s, feeds out_p)."""
            _, ps_ = ppps[b]
            pt, _ = ptst[b]
            at8 = heads.pop((b, h))[2]
            ps_ws = psum.tile([P, L], F32, name="ps_ws", tag="mm", bufs=MMB)
            for k in range(2):
                nc.tensor.matmul(
                    ps_ws[:],
                    ps_[:, 2 * k:2 * k + 2, h * HD:(h + 1) * HD],
                    at8[:, 2 * k:2 * k + 2, 0:L],
                    start=(k == 0),
                    stop=(k == 1),
                    perf_mode=DR,
                )
            pool = pt[:, NT, :]
            if h == 0:
                nc.vector.tensor_scalar_max(pool, ps_ws[:], 0.0)
            else:
                nc.vector.tensor_max(out=pool, in0=pool, in1=ps_ws[:])

        def stage_outs(b):
            pt, st = ptst.pop(b)
            ppps.pop(b)
            if not masks_trivial:
                _, _, pm_bc, sm_bc = masks.pop(b)
                # wp^T pools scale by sm_j (free dim j); ws^T pools by pm_i.
                nc.vector.tensor_mul(out=st[:, NT, :], in0=st[:, NT, :], in1=sm_bc[:])
                nc.vector.tensor_mul(out=pt[:, NT, :], in0=pt[:, NT, :], in1=pm_bc[:])
            for oi, (name_o, dst_d, lhs_t, w_o, bb) in enumerate((
                ("o_p", out_p, pt, w_fp_sb, "p"),
                ("o_s", out_s, st, w_fs_sb, "s"),
            )):
                for io in range(NT):
                    ps_mm = psum.tile([P, 512], F32, name="ps_out", tag="mm", bufs=MMB)
                    for co in range(NCT):
                        nc.tensor.matmul(
                            ps_mm[:],
                            lhs_t[:, co, io * P:(io + 1) * P],
                            w_o[:, co, :],
                            start=(co == 0),
                            stop=(co == NCT - 1),
                        )
                    o_sb = iopool.tile([P, D], BF16, name=name_o, tag=name_o, bufs=4)
                    if bias_trivial:
                        if (oi * NT + io) % 2 == 0:
                            nc.scalar.activation(out=o_sb[:], in_=ps_mm[:], func=RELU)
                        else:
                            nc.vector.tensor_scalar_max(o_sb[:], ps_mm[:], 0.0)
                    else:
                        bbt = bias_p_bc if bb == "p" else bias_s_bc
                        o32 = iopool.tile([P, D], F32, name="o32", tag="o32", bufs=2)
                        nc.vector.tensor_add(out=o32[:], in0=ps_mm[:], in1=bbt[:])
                        nc.vector.tensor_scalar_max(o_sb[:], o32[:], 0.0)
                    nc.gpsimd.dma_start(dst_d[b, io * P:(io + 1) * P, :], o_sb[:])

        # ---- unified cross-batch pipeline -------------------------------
        # Global step g: PW(g) | proj(dh) on steps 2,3 of each batch |
        # A(g-1) | at8(g-2) | WP/WS(g-3) | outs after each batch's last WS.
        # Projections sit at steps 2-3 so their weights (separate DMA
        # stream) have arrived; WP/WS skew 3 covers it (WP(b,0) at step 3).
        G = BPC * H

        def bh(g):
            return divmod(g, H)

        for g in range(G + 4):
            if g < G:
                stage_pw(*bh(g))
            if g < G and g % H in (2, 3):
                proj_stage(g // H, g % H - 2)
            if 0 <= g - 1 < G:
                stage_a(*bh(g - 1))
            if 0 <= g - 2 < G:
                stage_at8(*bh(g - 2))
            if 0 <= g - 3 < G:
                b3, h3 = bh(g - 3)
                stage_wp(b3, h3)
                stage_ws(b3, h3)
                if h3 == H - 1:
                    stage_outs(b3)


_PROGRAM_CACHE = {}


def _get_program(masks_trivial, bias_trivial):
    key = (masks_trivial, bias_trivial)
    if key not in _PROGRAM_CACHE:
        _PROGRAM_CACHE[key] = _build_program(masks_trivial, bias_trivial)
    return _PROGRAM_CACHE[key]


def kernel(
    primary, secondary, primary_mask, secondary_mask,
    W_aff, W_p, W_s, W_fp, b_fp, W_fs, b_fs,
    _trace=False,
):
    import ml_dtypes

    f32 = np.float32
    bf16 = ml_dtypes.bfloat16

    def qb(x):
        return np.asarray(x, f32).astype(bf16)

    # Host-side pre-transpose: every matmul consumes P/S with D on the
    # partition (contraction) axis, so ship (B, D, L) bf16 directly.
    primary_t = np.ascontiguousarray(
        qb(primary).transpose(0, 2, 1)
    )
    secondary_t = np.ascontiguousarray(
        qb(secondary).transpose(0, 2, 1)
    )
    primary_mask = np.ascontiguousarray(np.asarray(primary_mask, f32))
    secondary_mask = np.ascontiguousarray(np.asarray(secondary_mask, f32))

    weights = {
        "W_aff": qb(W_aff),
        "W_p": qb(W_p),
        "W_s": qb(W_s),
        "W_fp": qb(W_fp),
        "b_fp": np.ascontiguousarray(np.asarray(b_fp, f32)),
        "W_fs": qb(W_fs),
        "b_fs": np.ascontiguousarray(np.asarray(b_fs, f32)),
    }

    masks_trivial = bool(
        (primary_mask == 1.0).all() and (secondary_mask == 1.0).all()
    )
    bias_trivial = not (weights["b_fp"].any() or weights["b_fs"].any())

    nc = _get_program(masks_trivial, bias_trivial)

    in_maps = []
    for c in range(NCORES):
        sl = slice(c * BPC, (c + 1) * BPC)
        in_maps.append(
            {
                "primary_t": primary_t[sl],
                "secondary_t": secondary_t[sl],
                "primary_mask": primary_mask[sl],
                "secondary_mask": secondary_mask[sl],
                **weights,
            }
        )

    res = bass_utils.run_bass_kernel_spmd(
        nc, in_maps, core_ids=list(range(NCORES)), trace=_trace
    )
    out_p = np.concatenate(
        [np.asarray(r["out_p"], f32) for r in res.results], axis=0
    )
    out_s = np.concatenate(
        [np.asarray(r["out_s"], f32) for r in res.results], axis=0
    )
    if _trace:
        kernel.last_results = res
    return out_p, out_s
